# revision 5
# baseline (speedup 1.0000x reference)
"""Trainium2 Bass kernel for batched Bayesian linear regression (nn_BLR).

Math per task b (B=128 tasks, data-parallel over 8 NeuronCores, 16 tasks/core):
  G  = phiS^T phiS + P,  P = A A^T  (prior precision, shared)
  S  = G^{-1} kept in block-LDL factored form (never assembled):
         A128 = G[:128,:128] inverted by Newton-Schulz (fp32, J1 iters)
         U = A^{-1}B, Schur = C - B^T U inverted by Newton-Schulz (J2 iters)
  m  = S rhs with one iterative-refinement pass (restores backward-stable
       error structure; raw Newton inverses alone are not enough)
  mu = phiQ m
  spread_q = 1 + phi_q^T S phi_q  via the factored form on phiQ^T
  sig = spread*eps on the diagonal of [64,64] blocks (generated on device)
  nll partial sums (log spread, resid^2/(spread*eps)) reduced on host.
"""

import os
import sys
import types

import numpy as np

_TRN = "/opt/trn_rl_repo"
if os.path.isdir(_TRN) and _TRN not in sys.path:
    sys.path.insert(0, _TRN)

# NTFF profiling hook shim: bass_utils wants antenv.axon_hooks, which this
# image's antenv lacks. Provide it from trn_boot's ctypes implementation so
# trace=True yields exec_time_ns. Degrades to no-op when unavailable.
try:
    from antenv.axon_hooks import get_axon_ntff_profile_hook  # noqa: F401
except ImportError:
    _hook = None
    try:
        _bootdir = "/root/.axon_site/trn_agent_boot"
        if os.path.isdir(_bootdir):
            if _bootdir not in sys.path:
                sys.path.insert(0, _bootdir)
            import trn_boot  # type: ignore

            _so = "/opt/axon/libaxon_pjrt.so"
            if os.path.exists(_so):
                _hook = trn_boot._ntff_profile_via_ctypes(_so)
    except Exception:
        _hook = None
    _mod = types.ModuleType("antenv.axon_hooks")
    _mod.get_axon_ntff_profile_hook = lambda: _hook
    _mod.set_axon_ntff_profile_hook = lambda h: None
    sys.modules["antenv.axon_hooks"] = _mod

import concourse.bass as bass  # noqa: E402
import concourse.mybir as mybir  # noqa: E402
from concourse import bacc, tile  # noqa: E402
from concourse.bass_utils import run_bass_kernel_spmd  # noqa: E402

B, NS, NQ, DIN, DOUT = 128, 128, 512, 256, 64
NCORES = 8
BL = B // NCORES  # 16 tasks per core
J1, J2 = 16, 18  # Newton iterations for the two 128x128 pivots
F32 = mybir.dt.float32
ALU = mybir.AluOpType
ACTF = mybir.ActivationFunctionType
AXL = mybir.AxisListType
H = 128  # half of DIN; also the partition width


def _newton_inv(nc, cp, sp, pp, Dneg, i128, twoI, onescol, ones1row, one11, iters, tg):
    """Invert SPD D (given as Dneg = -D, [128,128] SBUF) by Newton-Schulz.

    X0 = I/||D||_F, X' = 2X + X @ (-D X). Returns SBUF tile with D^{-1}.
    tg: tag prefix for tile identity across tasks.
    """
    # ||D||_F^2 row partials via (Dneg*1)*Dneg with free-dim accumulate
    frocol = sp.tile([H, 1], F32, name=f"{tg}_frocol", tag=f"{tg}_frocol", bufs=2)
    scr = sp.tile([H, H], F32, name=f"{tg}_froscr", tag=f"{tg}_froscr", bufs=2)
    nc.vector.scalar_tensor_tensor(
        scr[:], Dneg[:], 1.0, Dneg[:], ALU.mult, ALU.mult, accum_out=frocol[:]
    )
    psf = pp.tile([1, 1], F32, name=f"{tg}_psf", tag="ps_tiny", bufs=2)
    nc.tensor.matmul(psf[:], onescol[:], frocol[:], start=True, stop=True)
    fro = sp.tile([1, 1], F32, name=f"{tg}_fro", tag=f"{tg}_fro", bufs=2)
    nc.scalar.activation(fro[:], psf[:], ACTF.Sqrt)  # ||D||_F
    alpha = sp.tile([1, 1], F32, name=f"{tg}_alpha", tag=f"{tg}_alpha", bufs=2)
    nc.vector.reciprocal(alpha[:], fro[:])
    psb = pp.tile([H, 1], F32, name=f"{tg}_psb", tag="ps_tiny", bufs=2)
    nc.tensor.matmul(psb[:], ones1row[:], alpha[:], start=True, stop=True)
    alphav = sp.tile([H, 1], F32, name=f"{tg}_alphav", tag=f"{tg}_alphav", bufs=2)
    nc.scalar.copy(alphav[:], psb[:])

    X = sp.tile([H, H], F32, name=f"{tg}_X0", tag=f"{tg}_X", bufs=3)
    nc.vector.tensor_scalar_mul(X[:], i128[:], alphav[:])
    for _ in range(iters):
        psY = pp.tile([H, H], F32, name=f"{tg}_psY", tag="ps", bufs=6)
        nc.tensor.matmul(psY[:], Dneg[:], X[:], start=True, stop=True)
        Yn = sp.tile([H, H], F32, name=f"{tg}_Yn", tag=f"{tg}_Yn", bufs=3)
        nc.scalar.copy(Yn[:], psY[:])  # Yn = -D X
        psX = pp.tile([H, H], F32, name=f"{tg}_psX", tag="ps", bufs=6)
        nc.tensor.matmul(psX[:], X[:], Yn[:], start=True, stop=False)
        nc.tensor.matmul(psX[:], i128[:], X[:], start=False, stop=False)
        nc.tensor.matmul(psX[:], X[:], i128[:], start=False, stop=True)
        Xn = sp.tile([H, H], F32, name=f"{tg}_Xn", tag=f"{tg}_X", bufs=3)
        # X' = X^T Yn + X + X^T  (== X(2I - DX) for symmetric X, but the
        # asymmetry rounding seed no longer doubles per iteration)
        nc.scalar.copy(Xn[:], psX[:])
        X = Xn
    return X


def _build():
    nc = bacc.Bacc("TRN2", target_bir_lowering=False, debug=False)

    phiS_d = nc.dram_tensor("phis", [BL, NS, DIN], F32, kind="ExternalInput")
    yS_d = nc.dram_tensor("ys", [BL, NS, DOUT], F32, kind="ExternalInput")
    phiQ_d = nc.dram_tensor("phiq", [BL, NQ, DIN], F32, kind="ExternalInput")
    yQ_d = nc.dram_tensor("yq", [BL, NQ, DOUT], F32, kind="ExternalInput")
    Ap_d = nc.dram_tensor("aprior", [DIN, DIN], F32, kind="ExternalInput")
    mp_d = nc.dram_tensor("mprior", [DIN, DOUT], F32, kind="ExternalInput")
    eps_d = nc.dram_tensor("sigeps", [1, 1], F32, kind="ExternalInput")
    i128_d = nc.dram_tensor("ceye", [H, H], F32, kind="ExternalInput")
    mask_d = nc.dram_tensor("cmask", [H, DOUT * DOUT], F32, kind="ExternalInput")

    mu_d = nc.dram_tensor("mu_out", [BL, NQ, DOUT], F32, kind="ExternalOutput")
    sig_d = nc.dram_tensor("sig_out", [BL, NQ, DOUT * DOUT], F32, kind="ExternalOutput")
    misc_d = nc.dram_tensor("misc_out", [1, 2], F32, kind="ExternalOutput")

    with tile.TileContext(nc) as tc:
        with (
            tc.tile_pool(name="cp", bufs=1) as cp,
            tc.tile_pool(name="sp", bufs=2) as sp,
            tc.tile_pool(name="pp", bufs=6, space="PSUM") as pp,
        ):
            # ---- constants ----
            i128 = cp.tile([H, H], F32)
            nc.sync.dma_start(i128[:], i128_d[:])
            mask = cp.tile([H, DOUT * DOUT], F32)
            nc.sync.dma_start(mask[:], mask_d[:])
            twoI = cp.tile([H, H], F32)
            nc.vector.tensor_scalar_mul(twoI[:], i128[:], 2.0)
            onescol = cp.tile([H, 1], F32)
            nc.vector.memset(onescol[:], 1.0)
            ones1row = cp.tile([1, H], F32)
            nc.vector.memset(ones1row[:], 1.0)
            one11 = cp.tile([1, 1], F32)
            nc.vector.memset(one11[:], 1.0)
            epssb = cp.tile([1, 1], F32)
            nc.sync.dma_start(epssb[:], eps_d[:])
            psb = pp.tile([H, 1], F32, tag="ps_tiny", bufs=2)
            nc.tensor.matmul(psb[:], ones1row[:], epssb[:], start=True, stop=True)
            epsvec = cp.tile([H, 1], F32)
            nc.scalar.copy(epsvec[:], psb[:])

            # ---- prior precision P = A A^T and mpn = P m_prior ----
            # load A row-halves, transpose quadrants into AT tiles [j-half, i(256)]
            Ar = []
            for ih in range(2):
                t = cp.tile([H, DIN], F32, name=f"Ar{ih}")
                nc.sync.dma_start(t[:], Ap_d[ih * H : (ih + 1) * H, :])
                Ar.append(t)
            AT = []
            for jh in range(2):
                t = cp.tile([H, DIN], F32, name=f"AT{jh}")
                for ih in range(2):
                    pst = pp.tile([H, H], F32, name=f"psAT{jh}{ih}", tag="ps", bufs=6)
                    nc.tensor.transpose(
                        pst[:], Ar[ih][:, jh * H : (jh + 1) * H], i128[:]
                    )
                    nc.scalar.copy(t[:, ih * H : (ih + 1) * H], pst[:])
                AT.append(t)
            Pt = []
            for ih in range(2):
                psP = pp.tile([H, DIN], F32, name=f"psP{ih}", tag="ps", bufs=6)
                for jh in range(2):
                    nc.tensor.matmul(
                        psP[:],
                        AT[jh][:, ih * H : (ih + 1) * H],
                        AT[jh][:],
                        start=(jh == 0),
                        stop=(jh == 1),
                    )
                t = cp.tile([H, DIN], F32, name=f"Pt{ih}")
                nc.scalar.copy(t[:], psP[:])
                Pt.append(t)
            mpt = []
            for jh in range(2):
                t = cp.tile([H, DOUT], F32, name=f"mpt{jh}")
                nc.sync.dma_start(t[:], mp_d[jh * H : (jh + 1) * H, :])
                mpt.append(t)
            mpn = []
            for ih in range(2):
                psm = pp.tile([H, DOUT], F32, name=f"psmpn{ih}", tag="ps", bufs=6)
                for jh in range(2):
                    nc.tensor.matmul(
                        psm[:],
                        Pt[jh][:, ih * H : (ih + 1) * H],
                        mpt[jh][:],
                        start=(jh == 0),
                        stop=(jh == 1),
                    )
                t = cp.tile([H, DOUT], F32, name=f"mpn{ih}")
                nc.scalar.copy(t[:], psm[:])
                mpn.append(t)

            # batched per-core stats tiles [128, 64]: col = t*4 + qchunk
            spread_all = cp.tile([H, 64], F32)
            spread_eps_all = cp.tile([H, 64], F32)
            resid2_all = cp.tile([H, 64], F32)

            # ---- per-task pipeline ----
            for t in range(BL):
                # Stage A: Gram + rhs
                phiS_t = sp.tile([NS, DIN], F32, tag="phis", bufs=3)
                nc.sync.dma_start(phiS_t[:], phiS_d[t])
                yS_t = sp.tile([NS, DOUT], F32, tag="ysup", bufs=3)
                nc.sync.dma_start(yS_t[:], yS_d[t])

                G = []
                Aneg = sp.tile([H, H], F32, tag="aneg", bufs=3)
                for ih in range(2):
                    psG = pp.tile([H, DIN], F32, name=f"psG{ih}", tag="ps", bufs=6)
                    nc.tensor.matmul(
                        psG[:],
                        phiS_t[:, ih * H : (ih + 1) * H],
                        phiS_t[:],
                        start=True,
                        stop=True,
                    )
                    g = sp.tile([H, DIN], F32, name=f"G{ih}", tag=f"g{ih}", bufs=3)
                    nc.vector.tensor_add(g[:], psG[:], Pt[ih][:])
                    if ih == 0:
                        # Aneg = -(psG + P)[:, :128]
                        nc.vector.scalar_tensor_tensor(
                            Aneg[:],
                            psG[:, 0:H],
                            -1.0,
                            Pt[0][:, 0:H],
                            ALU.mult,
                            ALU.subtract,
                        )
                    G.append(g)
                V = []
                for ih in range(2):
                    psR = pp.tile([H, DOUT], F32, name=f"psR{ih}", tag="ps", bufs=6)
                    nc.tensor.matmul(
                        psR[:],
                        phiS_t[:, ih * H : (ih + 1) * H],
                        yS_t[:],
                        start=True,
                        stop=True,
                    )
                    v = sp.tile([H, DOUT], F32, name=f"V{ih}", tag=f"v{ih}", bufs=3)
                    nc.vector.tensor_add(v[:], psR[:], mpn[ih][:])
                    V.append(v)

                # Stage B: invert A, build U/UT/Schur, invert Schur
                Ainv = _newton_inv(
                    nc, cp, sp, pp, Aneg, i128, twoI, onescol, ones1row, one11, J1, "na"
                )
                Bv = G[0][:, H:DIN]  # B block view [128, 128]
                psU = pp.tile([H, H], F32, tag="ps", bufs=6)
                nc.tensor.matmul(psU[:], Ainv[:], Bv, start=True, stop=True)
                U = sp.tile([H, H], F32, tag="u", bufs=3)
                nc.scalar.copy(U[:], psU[:])
                psUT = pp.tile([H, H], F32, tag="ps", bufs=6)
                nc.tensor.matmul(psUT[:], Bv, Ainv[:], start=True, stop=True)
                UT = sp.tile([H, H], F32, tag="ut", bufs=3)
                nc.scalar.copy(UT[:], psUT[:])
                psSc = pp.tile([H, H], F32, tag="ps", bufs=6)
                nc.tensor.matmul(psSc[:], Bv, U[:], start=True, stop=True)
                Scneg0 = sp.tile([H, H], F32, tag="scneg0", bufs=3)
                # Scneg = B^T U - C = -(C - B^T U), then symmetrized (the
                # Newton recurrence needs D^T = D bitwise)
                nc.vector.tensor_sub(Scneg0[:], psSc[:], G[1][:, H:DIN])
                psScT = pp.tile([H, H], F32, tag="ps", bufs=6)
                nc.tensor.transpose(psScT[:], Scneg0[:], i128[:])
                Sctmp = sp.tile([H, H], F32, tag="sctmp", bufs=3)
                nc.vector.tensor_add(Sctmp[:], Scneg0[:], psScT[:])
                Scneg = sp.tile([H, H], F32, tag="scneg", bufs=3)
                nc.vector.tensor_scalar_mul(Scneg[:], Sctmp[:], 0.5)
                Scinv = _newton_inv(
                    nc, cp, sp, pp, Scneg, i128, twoI, onescol, ones1row, one11, J2, "ns"
                )

                # Stage C: m = S rhs with one refinement pass, factored applies
                def apply_s(V1ap, V2ap, nm):
                    psT2 = pp.tile([H, DOUT], F32, name=f"psT2{nm}", tag="ps", bufs=6)
                    nc.tensor.matmul(psT2[:], U[:], V1ap, start=True, stop=True)
                    T2 = sp.tile([H, DOUT], F32, name=f"T2{nm}", tag=f"t2{nm}", bufs=2)
                    nc.vector.tensor_sub(T2[:], V2ap, psT2[:])
                    psZ1 = pp.tile([H, DOUT], F32, name=f"psZ1{nm}", tag="ps", bufs=6)
                    nc.tensor.matmul(psZ1[:], Ainv[:], V1ap, start=True, stop=True)
                    Z1 = sp.tile([H, DOUT], F32, name=f"Z1{nm}", tag=f"z1{nm}", bufs=2)
                    nc.scalar.copy(Z1[:], psZ1[:])
                    psZ2 = pp.tile([H, DOUT], F32, name=f"psZ2{nm}", tag="ps", bufs=6)
                    nc.tensor.matmul(psZ2[:], Scinv[:], T2[:], start=True, stop=True)
                    Z2 = sp.tile([H, DOUT], F32, name=f"Z2{nm}", tag=f"z2{nm}", bufs=2)
                    nc.scalar.copy(Z2[:], psZ2[:])
                    psR1 = pp.tile([H, DOUT], F32, name=f"psR1{nm}", tag="ps", bufs=6)
                    nc.tensor.matmul(psR1[:], UT[:], Z2[:], start=True, stop=True)
                    M1 = sp.tile([H, DOUT], F32, name=f"M1{nm}", tag=f"m1{nm}", bufs=2)
                    nc.vector.tensor_sub(M1[:], Z1[:], psR1[:])
                    return M1, Z2

                m1, m2 = apply_s(V[0][:], V[1][:], "a")
                mm = [m1, m2]
                R = []
                for ih in range(2):
                    psGm = pp.tile([H, DOUT], F32, name=f"psGm{ih}", tag="ps", bufs=6)
                    for jh in range(2):
                        nc.tensor.matmul(
                            psGm[:],
                            G[jh][:, ih * H : (ih + 1) * H],
                            mm[jh][:],
                            start=(jh == 0),
                            stop=(jh == 1),
                        )
                    r = sp.tile([H, DOUT], F32, name=f"R{ih}", tag=f"r{ih}", bufs=2)
                    nc.vector.tensor_sub(r[:], V[ih][:], psGm[:])
                    R.append(r)
                d1, d2 = apply_s(R[0][:], R[1][:], "b")
                mf1 = sp.tile([H, DOUT], F32, tag="mf1", bufs=2)
                nc.vector.tensor_add(mf1[:], m1[:], d1[:])
                mf2 = sp.tile([H, DOUT], F32, tag="mf2", bufs=2)
                nc.vector.tensor_add(mf2[:], m2[:], d2[:])
                mf = [mf1, mf2]

                # Stage D: phiQ transpose, mu/resid, spread
                natQ = sp.tile([H, 4 * DIN], F32, tag="natq", bufs=2)
                yQn = sp.tile([H, 4 * DOUT], F32, tag="yqn", bufs=2)
                for c in range(4):
                    nc.sync.dma_start(
                        natQ[:, c * DIN : (c + 1) * DIN],
                        phiQ_d[t, c * H : (c + 1) * H, :],
                    )
                    nc.sync.dma_start(
                        yQn[:, c * DOUT : (c + 1) * DOUT],
                        yQ_d[t, c * H : (c + 1) * H, :],
                    )
                F1 = sp.tile([H, NQ], F32, tag="f1", bufs=3)
                F2 = sp.tile([H, NQ], F32, tag="f2", bufs=3)
                for c in range(4):
                    for ih, Ft in ((0, F1), (1, F2)):
                        pst = pp.tile(
                            [H, H], F32, name=f"psFt{c}{ih}", tag="ps", bufs=6
                        )
                        nc.tensor.transpose(
                            pst[:],
                            natQ[:, c * DIN + ih * H : c * DIN + (ih + 1) * H],
                            i128[:],
                        )
                        nc.scalar.copy(Ft[:, c * H : (c + 1) * H], pst[:])

                mu_st = sp.tile([H, 4 * DOUT], F32, tag="must", bufs=2)
                sqscr = sp.tile([H, DOUT], F32, tag="sqscr", bufs=2)
                for c in range(4):
                    psMu = pp.tile([H, DOUT], F32, name=f"psMu{c}", tag="ps", bufs=6)
                    for ih in range(2):
                        nc.tensor.matmul(
                            psMu[:],
                            (F1 if ih == 0 else F2)[:, c * H : (c + 1) * H],
                            mf[ih][:],
                            start=(ih == 0),
                            stop=(ih == 1),
                        )
                    nc.scalar.copy(mu_st[:, c * DOUT : (c + 1) * DOUT], psMu[:])
                    resid = sp.tile([H, DOUT], F32, name=f"resid{c}", tag="resid", bufs=2)
                    nc.vector.tensor_sub(
                        resid[:], yQn[:, c * DOUT : (c + 1) * DOUT], psMu[:]
                    )
                    col = t * 4 + c
                    nc.scalar.activation(
                        sqscr[:],
                        resid[:],
                        ACTF.Square,
                        accum_out=resid2_all[:, col : col + 1],
                    )
                for c in range(4):
                    nc.sync.dma_start(
                        mu_d[t, c * H : (c + 1) * H, :],
                        mu_st[:, c * DOUT : (c + 1) * DOUT],
                    )

                # spread: z-form  s = (Ainv F1) . F1 + (Scinv H) . H,  H = F2 - U^T F1
                psH = pp.tile([H, NQ], F32, tag="ps", bufs=6)
                nc.tensor.matmul(psH[:], U[:], F1[:], start=True, stop=True)
                Ht = sp.tile([H, NQ], F32, tag="ht", bufs=2)
                nc.vector.tensor_sub(Ht[:], F2[:], psH[:])
                psY1 = pp.tile([H, NQ], F32, tag="ps", bufs=6)
                nc.tensor.matmul(psY1[:], Ainv[:], F1[:], start=True, stop=True)
                s1 = sp.tile([H, NQ], F32, tag="s1", bufs=2)
                nc.vector.tensor_mul(s1[:], psY1[:], F1[:])
                psY2 = pp.tile([H, NQ], F32, tag="ps", bufs=6)
                nc.tensor.matmul(psY2[:], Scinv[:], Ht[:], start=True, stop=True)
                s2 = sp.tile([H, NQ], F32, tag="s2", bufs=2)
                nc.vector.tensor_mul(s2[:], psY2[:], Ht[:])
                psSp = pp.tile([1, NQ], F32, tag="ps", bufs=6)
                nc.tensor.matmul(psSp[:], onescol[:], s1[:], start=True, stop=False)
                nc.tensor.matmul(psSp[:], onescol[:], s2[:], start=False, stop=True)
                sprow = sp.tile([1, NQ], F32, tag="sprow", bufs=2)
                nc.scalar.copy(sprow[:], psSp[:])
                for c in range(4):
                    col = t * 4 + c
                    psq = pp.tile([H, 1], F32, name=f"psq{c}", tag="ps_tiny", bufs=2)
                    nc.tensor.matmul(
                        psq[:],
                        sprow[:, c * H : (c + 1) * H],
                        one11[:],
                        start=True,
                        stop=True,
                    )
                    # spread = qSq + 1
                    nc.scalar.activation(
                        spread_all[:, col : col + 1], psq[:], ACTF.Copy, bias=1.0
                    )
                    nc.vector.tensor_scalar_mul(
                        spread_eps_all[:, col : col + 1],
                        spread_all[:, col : col + 1],
                        epsvec[:],
                    )
                    # sig block: [128 q, 64*64] = mask * spread_eps
                    sig_t = sp.tile([H, DOUT * DOUT], F32, name=f"sig{c}", tag="sig", bufs=3)
                    nc.vector.tensor_scalar_mul(
                        sig_t[:], mask[:], spread_eps_all[:, col : col + 1]
                    )
                    nc.sync.dma_start(sig_d[t, c * H : (c + 1) * H, :], sig_t[:])

            # ---- Stage E: nll partials ----
            logt = cp.tile([H, 64], F32)
            nc.scalar.activation(logt[:], spread_all[:], ACTF.Ln)
            recip = cp.tile([H, 64], F32)
            nc.vector.reciprocal(recip[:], spread_eps_all[:])
            quad = cp.tile([H, 64], F32)
            nc.vector.tensor_mul(quad[:], resid2_all[:], recip[:])
            lq = cp.tile([H, 2], F32)
            nc.vector.tensor_reduce(lq[:, 0:1], logt[:], AXL.X, ALU.add)
            nc.vector.tensor_reduce(lq[:, 1:2], quad[:], AXL.X, ALU.add)
            psF = pp.tile([1, 2], F32, tag="ps_tiny", bufs=2)
            nc.tensor.matmul(psF[:], onescol[:], lq[:], start=True, stop=True)
            misc = cp.tile([1, 2], F32)
            nc.scalar.copy(misc[:], psF[:])
            nc.sync.dma_start(misc_d[:], misc[:])

    nc.compile()
    return nc


_NC_CACHE = []


def _get_nc():
    if not _NC_CACHE:
        _NC_CACHE.append(_build())
    return _NC_CACHE[0]


def kernel(phi_support, y_support, phi_query, y_query, m_prior, S_inv_prior_asym, sig_eps):
    f = np.float32
    phi_support = np.ascontiguousarray(phi_support, dtype=f)
    y_support = np.ascontiguousarray(y_support, dtype=f)
    phi_query = np.ascontiguousarray(phi_query, dtype=f)
    y_query = np.ascontiguousarray(y_query, dtype=f)
    m_prior = np.ascontiguousarray(m_prior, dtype=f)
    A = np.ascontiguousarray(S_inv_prior_asym, dtype=f)
    eps = np.asarray(sig_eps, dtype=f).reshape(1, 1)

    eye = np.eye(H, dtype=f)
    mask = np.zeros((H, DOUT * DOUT), dtype=f)
    mask[:, :: DOUT + 1] = 1.0  # flattened 64x64 identity pattern per partition

    in_maps = []
    for c in range(NCORES):
        s = slice(c * BL, (c + 1) * BL)
        in_maps.append(
            {
                "phis": phi_support[s],
                "ys": y_support[s],
                "phiq": phi_query[s],
                "yq": y_query[s],
                "aprior": A,
                "mprior": m_prior,
                "sigeps": eps,
                "ceye": eye,
                "cmask": mask,
            }
        )

    nc = _get_nc()
    res = run_bass_kernel_spmd(nc, in_maps, core_ids=list(range(NCORES)), trace=True)
    if res.exec_time_ns is not None:
        print(f"HW exec time: {res.exec_time_ns} ns")
        kernel.last_exec_time_ns = res.exec_time_ns

    mu = np.empty((B, NQ, DOUT), dtype=f)
    sig = np.empty((B, NQ, DOUT, DOUT), dtype=f)
    s_log = 0.0
    s_quad = 0.0
    for c in range(NCORES):
        out = res.results[c]
        mu[c * BL : (c + 1) * BL] = out["mu_out"]
        sig[c * BL : (c + 1) * BL] = out["sig_out"].reshape(BL, NQ, DOUT, DOUT)
        s_log += float(out["misc_out"][0, 0])
        s_quad += float(out["misc_out"][0, 1])
    BQ = B * NQ
    nll = np.float32(DOUT * (s_log / BQ + np.log(float(eps[0, 0]))) + s_quad / BQ)
    return mu, sig, nll


# revision 6
# speedup vs baseline: 1.2715x; 1.2715x over previous
"""Trainium2 Bass kernel for batched Bayesian linear regression (nn_BLR).

Math per task b (B=128 tasks, data-parallel over 8 NeuronCores, 16 tasks/core):
  G  = phiS^T phiS + P,  P = A A^T  (prior precision, shared)
  S  = G^{-1} kept in block-LDL factored form (never assembled):
         A128 = G[:128,:128] inverted by Newton-Schulz (fp32, J1 iters)
         U = A^{-1}B, Schur = C - B^T U inverted by Newton-Schulz (J2 iters)
  m  = S rhs with one iterative-refinement pass (restores backward-stable
       error structure; raw Newton inverses alone are not enough)
  mu = phiQ m
  spread_q = 1 + phi_q^T S phi_q  via the factored form on phiQ^T
  sig = spread*eps on the diagonal of [64,64] blocks (generated on device)
  nll partial sums (log spread, resid^2/(spread*eps)) reduced on host.
"""

import os
import sys
import types

import numpy as np

_TRN = "/opt/trn_rl_repo"
if os.path.isdir(_TRN) and _TRN not in sys.path:
    sys.path.insert(0, _TRN)

# NTFF profiling hook shim: bass_utils wants antenv.axon_hooks, which this
# image's antenv lacks. Provide it from trn_boot's ctypes implementation so
# trace=True yields exec_time_ns. Degrades to no-op when unavailable.
try:
    from antenv.axon_hooks import get_axon_ntff_profile_hook  # noqa: F401
except ImportError:
    _hook = None
    try:
        _bootdir = "/root/.axon_site/trn_agent_boot"
        if os.path.isdir(_bootdir):
            if _bootdir not in sys.path:
                sys.path.insert(0, _bootdir)
            import trn_boot  # type: ignore

            _so = "/opt/axon/libaxon_pjrt.so"
            if os.path.exists(_so):
                _hook = trn_boot._ntff_profile_via_ctypes(_so)
    except Exception:
        _hook = None
    _mod = types.ModuleType("antenv.axon_hooks")
    _mod.get_axon_ntff_profile_hook = lambda: _hook
    _mod.set_axon_ntff_profile_hook = lambda h: None
    sys.modules["antenv.axon_hooks"] = _mod

import concourse.bass as bass  # noqa: E402
import concourse.mybir as mybir  # noqa: E402
from concourse import bacc, tile  # noqa: E402
from concourse.bass_utils import run_bass_kernel_spmd  # noqa: E402

B, NS, NQ, DIN, DOUT = 128, 128, 512, 256, 64
NCORES = 8
BL = B // NCORES  # 16 tasks per core
J1, J2 = 15, 17  # Newton iterations for the two 128x128 pivots
F32 = mybir.dt.float32
ALU = mybir.AluOpType
ACTF = mybir.ActivationFunctionType
AXL = mybir.AxisListType
H = 128  # half of DIN; also the partition width


def _newton_inv(nc, cp, sp, pp, Dneg, i128, twoI, onescol, ones1row, one11, iters, tg):
    """Invert SPD D (given as Dneg = -D, [128,128] SBUF) by Newton-Schulz.

    X0 = I/||D||_F, X' = 2X + X @ (-D X). Returns SBUF tile with D^{-1}.
    tg: tag prefix for tile identity across tasks.
    """
    # ||D||_F^2 row partials via (Dneg*1)*Dneg with free-dim accumulate
    frocol = sp.tile([H, 1], F32, name=f"{tg}_frocol", tag=f"{tg}_frocol", bufs=2)
    scr = sp.tile([H, H], F32, name=f"{tg}_froscr", tag=f"{tg}_froscr", bufs=2)
    nc.vector.scalar_tensor_tensor(
        scr[:], Dneg[:], 1.0, Dneg[:], ALU.mult, ALU.mult, accum_out=frocol[:]
    )
    psf = pp.tile([1, 1], F32, name=f"{tg}_psf", tag="ps_tiny", bufs=2)
    nc.tensor.matmul(psf[:], onescol[:], frocol[:], start=True, stop=True)
    fro = sp.tile([1, 1], F32, name=f"{tg}_fro", tag=f"{tg}_fro", bufs=2)
    nc.scalar.activation(fro[:], psf[:], ACTF.Sqrt)  # ||D||_F
    alpha = sp.tile([1, 1], F32, name=f"{tg}_alpha", tag=f"{tg}_alpha", bufs=2)
    nc.vector.reciprocal(alpha[:], fro[:])
    psb = pp.tile([H, 1], F32, name=f"{tg}_psb", tag="ps_tiny", bufs=2)
    nc.tensor.matmul(psb[:], ones1row[:], alpha[:], start=True, stop=True)
    alphav = sp.tile([H, 1], F32, name=f"{tg}_alphav", tag=f"{tg}_alphav", bufs=2)
    nc.scalar.copy(alphav[:], psb[:])

    X = sp.tile([H, H], F32, name=f"{tg}_X0", tag=f"{tg}_X", bufs=3)
    nc.vector.tensor_scalar_mul(X[:], i128[:], alphav[:])
    for k in range(iters):
        psY = pp.tile([H, H], F32, name=f"{tg}_psY", tag="ps", bufs=6)
        nc.tensor.matmul(psY[:], Dneg[:], X[:], start=True, stop=True)
        Yn = sp.tile([H, H], F32, name=f"{tg}_Yn", tag=f"{tg}_Yn", bufs=3)
        nc.scalar.copy(Yn[:], psY[:])  # Yn = -D X
        psX = pp.tile([H, H], F32, name=f"{tg}_psX", tag="ps", bufs=6)
        nc.tensor.matmul(psX[:], X[:], Yn[:], start=True, stop=True)
        Xn = sp.tile([H, H], F32, name=f"{tg}_Xn", tag=f"{tg}_X", bufs=3)
        # X' = 2X + X^T Yn; the asymmetry seed doubles per iter, so
        # resymmetrize every ~6 iterations (cheap, keeps e ~ eps)
        nc.vector.scalar_tensor_tensor(
            Xn[:], X[:], 2.0, psX[:], ALU.mult, ALU.add
        )
        X = Xn
        if k in (6, 12):
            psT = pp.tile([H, H], F32, name=f"{tg}_psT", tag="ps", bufs=6)
            nc.tensor.transpose(psT[:], X[:], i128[:])
            Xs = sp.tile([H, H], F32, name=f"{tg}_Xs", tag=f"{tg}_X", bufs=3)
            nc.vector.tensor_add(Xs[:], X[:], psT[:])
            Xs2 = sp.tile([H, H], F32, name=f"{tg}_Xs2", tag=f"{tg}_X", bufs=3)
            nc.vector.tensor_scalar_mul(Xs2[:], Xs[:], 0.5)
            X = Xs2
    return X


def _build():
    nc = bacc.Bacc("TRN2", target_bir_lowering=False, debug=False)

    phiS_d = nc.dram_tensor("phis", [BL, NS, DIN], F32, kind="ExternalInput")
    yS_d = nc.dram_tensor("ys", [BL, NS, DOUT], F32, kind="ExternalInput")
    phiQ_d = nc.dram_tensor("phiq", [BL, NQ, DIN], F32, kind="ExternalInput")
    yQ_d = nc.dram_tensor("yq", [BL, NQ, DOUT], F32, kind="ExternalInput")
    Ap_d = nc.dram_tensor("aprior", [DIN, DIN], F32, kind="ExternalInput")
    mp_d = nc.dram_tensor("mprior", [DIN, DOUT], F32, kind="ExternalInput")
    eps_d = nc.dram_tensor("sigeps", [1, 1], F32, kind="ExternalInput")
    i128_d = nc.dram_tensor("ceye", [H, H], F32, kind="ExternalInput")
    mask_d = nc.dram_tensor("cmask", [H, DOUT * DOUT], F32, kind="ExternalInput")

    mu_d = nc.dram_tensor("mu_out", [BL, NQ, DOUT], F32, kind="ExternalOutput")
    sig_d = nc.dram_tensor("sig_out", [BL, NQ, DOUT * DOUT], F32, kind="ExternalOutput")
    misc_d = nc.dram_tensor("misc_out", [1, 2], F32, kind="ExternalOutput")

    with tile.TileContext(nc) as tc:
        with (
            tc.tile_pool(name="cp", bufs=1) as cp,
            tc.tile_pool(name="sp", bufs=2) as sp,
            tc.tile_pool(name="pp", bufs=6, space="PSUM") as pp,
        ):
            # ---- constants ----
            i128 = cp.tile([H, H], F32)
            nc.sync.dma_start(i128[:], i128_d[:])
            mask = cp.tile([H, DOUT * DOUT], F32)
            nc.sync.dma_start(mask[:], mask_d[:])
            twoI = cp.tile([H, H], F32)
            nc.vector.tensor_scalar_mul(twoI[:], i128[:], 2.0)
            onescol = cp.tile([H, 1], F32)
            nc.vector.memset(onescol[:], 1.0)
            ones1row = cp.tile([1, H], F32)
            nc.vector.memset(ones1row[:], 1.0)
            one11 = cp.tile([1, 1], F32)
            nc.vector.memset(one11[:], 1.0)
            epssb = cp.tile([1, 1], F32)
            nc.sync.dma_start(epssb[:], eps_d[:])
            psb = pp.tile([H, 1], F32, tag="ps_tiny", bufs=2)
            nc.tensor.matmul(psb[:], ones1row[:], epssb[:], start=True, stop=True)
            epsvec = cp.tile([H, 1], F32)
            nc.scalar.copy(epsvec[:], psb[:])

            # ---- prior precision P = A A^T and mpn = P m_prior ----
            # load A row-halves, transpose quadrants into AT tiles [j-half, i(256)]
            Ar = []
            for ih in range(2):
                t = cp.tile([H, DIN], F32, name=f"Ar{ih}")
                nc.sync.dma_start(t[:], Ap_d[ih * H : (ih + 1) * H, :])
                Ar.append(t)
            AT = []
            for jh in range(2):
                t = cp.tile([H, DIN], F32, name=f"AT{jh}")
                for ih in range(2):
                    pst = pp.tile([H, H], F32, name=f"psAT{jh}{ih}", tag="ps", bufs=6)
                    nc.tensor.transpose(
                        pst[:], Ar[ih][:, jh * H : (jh + 1) * H], i128[:]
                    )
                    nc.scalar.copy(t[:, ih * H : (ih + 1) * H], pst[:])
                AT.append(t)
            Pt = []
            for ih in range(2):
                psP = pp.tile([H, DIN], F32, name=f"psP{ih}", tag="ps", bufs=6)
                for jh in range(2):
                    nc.tensor.matmul(
                        psP[:],
                        AT[jh][:, ih * H : (ih + 1) * H],
                        AT[jh][:],
                        start=(jh == 0),
                        stop=(jh == 1),
                    )
                t = cp.tile([H, DIN], F32, name=f"Pt{ih}")
                nc.scalar.copy(t[:], psP[:])
                Pt.append(t)
            mpt = []
            for jh in range(2):
                t = cp.tile([H, DOUT], F32, name=f"mpt{jh}")
                nc.sync.dma_start(t[:], mp_d[jh * H : (jh + 1) * H, :])
                mpt.append(t)
            mpn = []
            for ih in range(2):
                psm = pp.tile([H, DOUT], F32, name=f"psmpn{ih}", tag="ps", bufs=6)
                for jh in range(2):
                    nc.tensor.matmul(
                        psm[:],
                        Pt[jh][:, ih * H : (ih + 1) * H],
                        mpt[jh][:],
                        start=(jh == 0),
                        stop=(jh == 1),
                    )
                t = cp.tile([H, DOUT], F32, name=f"mpn{ih}")
                nc.scalar.copy(t[:], psm[:])
                mpn.append(t)

            # batched per-core stats tiles [128, 64]: col = t*4 + qchunk
            spread_all = cp.tile([H, 64], F32)
            spread_eps_all = cp.tile([H, 64], F32)
            resid2_all = cp.tile([H, 64], F32)

            # ---- per-task pipeline ----
            for t in range(BL):
                # Stage A: Gram + rhs
                phiS_t = sp.tile([NS, DIN], F32, tag="phis", bufs=3)
                nc.sync.dma_start(phiS_t[:], phiS_d[t])
                yS_t = sp.tile([NS, DOUT], F32, tag="ysup", bufs=3)
                nc.sync.dma_start(yS_t[:], yS_d[t])

                G = []
                Aneg = sp.tile([H, H], F32, tag="aneg", bufs=3)
                for ih in range(2):
                    psG = pp.tile([H, DIN], F32, name=f"psG{ih}", tag="ps", bufs=6)
                    nc.tensor.matmul(
                        psG[:],
                        phiS_t[:, ih * H : (ih + 1) * H],
                        phiS_t[:],
                        start=True,
                        stop=True,
                    )
                    g = sp.tile([H, DIN], F32, name=f"G{ih}", tag=f"g{ih}", bufs=3)
                    nc.vector.tensor_add(g[:], psG[:], Pt[ih][:])
                    if ih == 0:
                        # Aneg = -(psG + P)[:, :128]
                        nc.vector.scalar_tensor_tensor(
                            Aneg[:],
                            psG[:, 0:H],
                            -1.0,
                            Pt[0][:, 0:H],
                            ALU.mult,
                            ALU.subtract,
                        )
                    G.append(g)
                V = []
                for ih in range(2):
                    psR = pp.tile([H, DOUT], F32, name=f"psR{ih}", tag="ps", bufs=6)
                    nc.tensor.matmul(
                        psR[:],
                        phiS_t[:, ih * H : (ih + 1) * H],
                        yS_t[:],
                        start=True,
                        stop=True,
                    )
                    v = sp.tile([H, DOUT], F32, name=f"V{ih}", tag=f"v{ih}", bufs=3)
                    nc.vector.tensor_add(v[:], psR[:], mpn[ih][:])
                    V.append(v)

                # Stage B: invert A, build U/UT/Schur, invert Schur
                Ainv = _newton_inv(
                    nc, cp, sp, pp, Aneg, i128, twoI, onescol, ones1row, one11, J1, "na"
                )
                Bv = G[0][:, H:DIN]  # B block view [128, 128]
                psU = pp.tile([H, H], F32, tag="ps", bufs=6)
                nc.tensor.matmul(psU[:], Ainv[:], Bv, start=True, stop=True)
                U = sp.tile([H, H], F32, tag="u", bufs=3)
                nc.scalar.copy(U[:], psU[:])
                psUT = pp.tile([H, H], F32, tag="ps", bufs=6)
                nc.tensor.matmul(psUT[:], Bv, Ainv[:], start=True, stop=True)
                UT = sp.tile([H, H], F32, tag="ut", bufs=3)
                nc.scalar.copy(UT[:], psUT[:])
                psSc = pp.tile([H, H], F32, tag="ps", bufs=6)
                nc.tensor.matmul(psSc[:], Bv, U[:], start=True, stop=True)
                Scneg0 = sp.tile([H, H], F32, tag="scneg0", bufs=3)
                # Scneg = B^T U - C = -(C - B^T U), then symmetrized (the
                # Newton recurrence needs D^T = D bitwise)
                nc.vector.tensor_sub(Scneg0[:], psSc[:], G[1][:, H:DIN])
                psScT = pp.tile([H, H], F32, tag="ps", bufs=6)
                nc.tensor.transpose(psScT[:], Scneg0[:], i128[:])
                Sctmp = sp.tile([H, H], F32, tag="sctmp", bufs=3)
                nc.vector.tensor_add(Sctmp[:], Scneg0[:], psScT[:])
                Scneg = sp.tile([H, H], F32, tag="scneg", bufs=3)
                nc.vector.tensor_scalar_mul(Scneg[:], Sctmp[:], 0.5)
                Scinv = _newton_inv(
                    nc, cp, sp, pp, Scneg, i128, twoI, onescol, ones1row, one11, J2, "ns"
                )

                # Stage C: m = S rhs with one refinement pass, factored applies
                def apply_s(V1ap, V2ap, nm):
                    psT2 = pp.tile([H, DOUT], F32, name=f"psT2{nm}", tag="ps", bufs=6)
                    nc.tensor.matmul(psT2[:], U[:], V1ap, start=True, stop=True)
                    T2 = sp.tile([H, DOUT], F32, name=f"T2{nm}", tag=f"t2{nm}", bufs=2)
                    nc.vector.tensor_sub(T2[:], V2ap, psT2[:])
                    psZ1 = pp.tile([H, DOUT], F32, name=f"psZ1{nm}", tag="ps", bufs=6)
                    nc.tensor.matmul(psZ1[:], Ainv[:], V1ap, start=True, stop=True)
                    Z1 = sp.tile([H, DOUT], F32, name=f"Z1{nm}", tag=f"z1{nm}", bufs=2)
                    nc.scalar.copy(Z1[:], psZ1[:])
                    psZ2 = pp.tile([H, DOUT], F32, name=f"psZ2{nm}", tag="ps", bufs=6)
                    nc.tensor.matmul(psZ2[:], Scinv[:], T2[:], start=True, stop=True)
                    Z2 = sp.tile([H, DOUT], F32, name=f"Z2{nm}", tag=f"z2{nm}", bufs=2)
                    nc.scalar.copy(Z2[:], psZ2[:])
                    psR1 = pp.tile([H, DOUT], F32, name=f"psR1{nm}", tag="ps", bufs=6)
                    nc.tensor.matmul(psR1[:], UT[:], Z2[:], start=True, stop=True)
                    M1 = sp.tile([H, DOUT], F32, name=f"M1{nm}", tag=f"m1{nm}", bufs=2)
                    nc.vector.tensor_sub(M1[:], Z1[:], psR1[:])
                    return M1, Z2

                m1, m2 = apply_s(V[0][:], V[1][:], "a")
                mm = [m1, m2]
                R = []
                for ih in range(2):
                    psGm = pp.tile([H, DOUT], F32, name=f"psGm{ih}", tag="ps", bufs=6)
                    for jh in range(2):
                        nc.tensor.matmul(
                            psGm[:],
                            G[jh][:, ih * H : (ih + 1) * H],
                            mm[jh][:],
                            start=(jh == 0),
                            stop=(jh == 1),
                        )
                    r = sp.tile([H, DOUT], F32, name=f"R{ih}", tag=f"r{ih}", bufs=2)
                    nc.vector.tensor_sub(r[:], V[ih][:], psGm[:])
                    R.append(r)
                d1, d2 = apply_s(R[0][:], R[1][:], "b")
                mf1 = sp.tile([H, DOUT], F32, tag="mf1", bufs=2)
                nc.vector.tensor_add(mf1[:], m1[:], d1[:])
                mf2 = sp.tile([H, DOUT], F32, tag="mf2", bufs=2)
                nc.vector.tensor_add(mf2[:], m2[:], d2[:])
                mf = [mf1, mf2]

                # Stage D: phiQ transpose, mu/resid, spread
                natQ = sp.tile([H, 4 * DIN], F32, tag="natq", bufs=2)
                yQn = sp.tile([H, 4 * DOUT], F32, tag="yqn", bufs=2)
                for c in range(4):
                    nc.sync.dma_start(
                        natQ[:, c * DIN : (c + 1) * DIN],
                        phiQ_d[t, c * H : (c + 1) * H, :],
                    )
                    nc.sync.dma_start(
                        yQn[:, c * DOUT : (c + 1) * DOUT],
                        yQ_d[t, c * H : (c + 1) * H, :],
                    )
                F1 = sp.tile([H, NQ], F32, tag="f1", bufs=3)
                F2 = sp.tile([H, NQ], F32, tag="f2", bufs=3)
                for c in range(4):
                    for ih, Ft in ((0, F1), (1, F2)):
                        pst = pp.tile(
                            [H, H], F32, name=f"psFt{c}{ih}", tag="ps", bufs=6
                        )
                        nc.tensor.transpose(
                            pst[:],
                            natQ[:, c * DIN + ih * H : c * DIN + (ih + 1) * H],
                            i128[:],
                        )
                        nc.scalar.copy(Ft[:, c * H : (c + 1) * H], pst[:])

                mu_st = sp.tile([H, 4 * DOUT], F32, tag="must", bufs=2)
                sqscr = sp.tile([H, DOUT], F32, tag="sqscr", bufs=2)
                for c in range(4):
                    psMu = pp.tile([H, DOUT], F32, name=f"psMu{c}", tag="ps", bufs=6)
                    for ih in range(2):
                        nc.tensor.matmul(
                            psMu[:],
                            (F1 if ih == 0 else F2)[:, c * H : (c + 1) * H],
                            mf[ih][:],
                            start=(ih == 0),
                            stop=(ih == 1),
                        )
                    nc.scalar.copy(mu_st[:, c * DOUT : (c + 1) * DOUT], psMu[:])
                    resid = sp.tile([H, DOUT], F32, name=f"resid{c}", tag="resid", bufs=2)
                    nc.vector.tensor_sub(
                        resid[:], yQn[:, c * DOUT : (c + 1) * DOUT], psMu[:]
                    )
                    col = t * 4 + c
                    nc.scalar.activation(
                        sqscr[:],
                        resid[:],
                        ACTF.Square,
                        accum_out=resid2_all[:, col : col + 1],
                    )
                for c in range(4):
                    nc.sync.dma_start(
                        mu_d[t, c * H : (c + 1) * H, :],
                        mu_st[:, c * DOUT : (c + 1) * DOUT],
                    )

                # spread: z-form  s = (Ainv F1) . F1 + (Scinv H) . H,  H = F2 - U^T F1
                psH = pp.tile([H, NQ], F32, tag="ps", bufs=6)
                nc.tensor.matmul(psH[:], U[:], F1[:], start=True, stop=True)
                Ht = sp.tile([H, NQ], F32, tag="ht", bufs=2)
                nc.vector.tensor_sub(Ht[:], F2[:], psH[:])
                psY1 = pp.tile([H, NQ], F32, tag="ps", bufs=6)
                nc.tensor.matmul(psY1[:], Ainv[:], F1[:], start=True, stop=True)
                s1 = sp.tile([H, NQ], F32, tag="s1", bufs=2)
                nc.vector.tensor_mul(s1[:], psY1[:], F1[:])
                psY2 = pp.tile([H, NQ], F32, tag="ps", bufs=6)
                nc.tensor.matmul(psY2[:], Scinv[:], Ht[:], start=True, stop=True)
                s2 = sp.tile([H, NQ], F32, tag="s2", bufs=2)
                nc.vector.tensor_mul(s2[:], psY2[:], Ht[:])
                psSp = pp.tile([1, NQ], F32, tag="ps", bufs=6)
                nc.tensor.matmul(psSp[:], onescol[:], s1[:], start=True, stop=False)
                nc.tensor.matmul(psSp[:], onescol[:], s2[:], start=False, stop=True)
                sprow = sp.tile([1, NQ], F32, tag="sprow", bufs=2)
                nc.scalar.copy(sprow[:], psSp[:])
                for c in range(4):
                    col = t * 4 + c
                    psq = pp.tile([H, 1], F32, name=f"psq{c}", tag="ps_tiny", bufs=2)
                    nc.tensor.matmul(
                        psq[:],
                        sprow[:, c * H : (c + 1) * H],
                        one11[:],
                        start=True,
                        stop=True,
                    )
                    # spread = qSq + 1
                    nc.scalar.activation(
                        spread_all[:, col : col + 1], psq[:], ACTF.Copy, bias=1.0
                    )
                    nc.vector.tensor_scalar_mul(
                        spread_eps_all[:, col : col + 1],
                        spread_all[:, col : col + 1],
                        epsvec[:],
                    )
                    # sig block: [128 q, 64*64] = mask * spread_eps
                    sig_t = sp.tile([H, DOUT * DOUT], F32, name=f"sig{c}", tag="sig", bufs=3)
                    nc.vector.tensor_scalar_mul(
                        sig_t[:], mask[:], spread_eps_all[:, col : col + 1]
                    )
                    nc.sync.dma_start(sig_d[t, c * H : (c + 1) * H, :], sig_t[:])

            # ---- Stage E: nll partials ----
            logt = cp.tile([H, 64], F32)
            nc.scalar.activation(logt[:], spread_all[:], ACTF.Ln)
            recip = cp.tile([H, 64], F32)
            nc.vector.reciprocal(recip[:], spread_eps_all[:])
            quad = cp.tile([H, 64], F32)
            nc.vector.tensor_mul(quad[:], resid2_all[:], recip[:])
            lq = cp.tile([H, 2], F32)
            nc.vector.tensor_reduce(lq[:, 0:1], logt[:], AXL.X, ALU.add)
            nc.vector.tensor_reduce(lq[:, 1:2], quad[:], AXL.X, ALU.add)
            psF = pp.tile([1, 2], F32, tag="ps_tiny", bufs=2)
            nc.tensor.matmul(psF[:], onescol[:], lq[:], start=True, stop=True)
            misc = cp.tile([1, 2], F32)
            nc.scalar.copy(misc[:], psF[:])
            nc.sync.dma_start(misc_d[:], misc[:])

    nc.compile()
    return nc


_NC_CACHE = []


def _get_nc():
    if not _NC_CACHE:
        _NC_CACHE.append(_build())
    return _NC_CACHE[0]


def kernel(phi_support, y_support, phi_query, y_query, m_prior, S_inv_prior_asym, sig_eps):
    f = np.float32
    phi_support = np.ascontiguousarray(phi_support, dtype=f)
    y_support = np.ascontiguousarray(y_support, dtype=f)
    phi_query = np.ascontiguousarray(phi_query, dtype=f)
    y_query = np.ascontiguousarray(y_query, dtype=f)
    m_prior = np.ascontiguousarray(m_prior, dtype=f)
    A = np.ascontiguousarray(S_inv_prior_asym, dtype=f)
    eps = np.asarray(sig_eps, dtype=f).reshape(1, 1)

    eye = np.eye(H, dtype=f)
    mask = np.zeros((H, DOUT * DOUT), dtype=f)
    mask[:, :: DOUT + 1] = 1.0  # flattened 64x64 identity pattern per partition

    in_maps = []
    for c in range(NCORES):
        s = slice(c * BL, (c + 1) * BL)
        in_maps.append(
            {
                "phis": phi_support[s],
                "ys": y_support[s],
                "phiq": phi_query[s],
                "yq": y_query[s],
                "aprior": A,
                "mprior": m_prior,
                "sigeps": eps,
                "ceye": eye,
                "cmask": mask,
            }
        )

    nc = _get_nc()
    res = run_bass_kernel_spmd(nc, in_maps, core_ids=list(range(NCORES)), trace=True)
    if res.exec_time_ns is not None:
        print(f"HW exec time: {res.exec_time_ns} ns")
        kernel.last_exec_time_ns = res.exec_time_ns

    mu = np.empty((B, NQ, DOUT), dtype=f)
    sig = np.empty((B, NQ, DOUT, DOUT), dtype=f)
    s_log = 0.0
    s_quad = 0.0
    for c in range(NCORES):
        out = res.results[c]
        mu[c * BL : (c + 1) * BL] = out["mu_out"]
        sig[c * BL : (c + 1) * BL] = out["sig_out"].reshape(BL, NQ, DOUT, DOUT)
        s_log += float(out["misc_out"][0, 0])
        s_quad += float(out["misc_out"][0, 1])
    BQ = B * NQ
    nll = np.float32(DOUT * (s_log / BQ + np.log(float(eps[0, 0]))) + s_quad / BQ)
    return mu, sig, nll


# revision 7
# speedup vs baseline: 1.2871x; 1.0123x over previous
"""Trainium2 Bass kernel for batched Bayesian linear regression (nn_BLR).

Math per task b (B=128 tasks, data-parallel over 8 NeuronCores, 16 tasks/core):
  G  = phiS^T phiS + P,  P = A A^T  (prior precision, shared)
  S  = G^{-1} kept in block-LDL factored form (never assembled):
         A128 = G[:128,:128] inverted by Newton-Schulz (fp32, J1 iters)
         U = A^{-1}B, Schur = C - B^T U inverted by Newton-Schulz (J2 iters)
  m  = S rhs with one iterative-refinement pass (restores backward-stable
       error structure; raw Newton inverses alone are not enough)
  mu = phiQ m
  spread_q = 1 + phi_q^T S phi_q  via the factored form on phiQ^T
  sig = spread*eps on the diagonal of [64,64] blocks (generated on device)
  nll partial sums (log spread, resid^2/(spread*eps)) reduced on host.
"""

import os
import sys
import types

import numpy as np

_TRN = "/opt/trn_rl_repo"
if os.path.isdir(_TRN) and _TRN not in sys.path:
    sys.path.insert(0, _TRN)

# NTFF profiling hook shim: bass_utils wants antenv.axon_hooks, which this
# image's antenv lacks. Provide it from trn_boot's ctypes implementation so
# trace=True yields exec_time_ns. Degrades to no-op when unavailable.
try:
    from antenv.axon_hooks import get_axon_ntff_profile_hook  # noqa: F401
except ImportError:
    _hook = None
    try:
        _bootdir = "/root/.axon_site/trn_agent_boot"
        if os.path.isdir(_bootdir):
            if _bootdir not in sys.path:
                sys.path.insert(0, _bootdir)
            import trn_boot  # type: ignore

            _so = "/opt/axon/libaxon_pjrt.so"
            if os.path.exists(_so):
                _hook = trn_boot._ntff_profile_via_ctypes(_so)
    except Exception:
        _hook = None
    _mod = types.ModuleType("antenv.axon_hooks")
    _mod.get_axon_ntff_profile_hook = lambda: _hook
    _mod.set_axon_ntff_profile_hook = lambda h: None
    sys.modules["antenv.axon_hooks"] = _mod

import concourse.bass as bass  # noqa: E402
import concourse.mybir as mybir  # noqa: E402
from concourse import bacc, tile  # noqa: E402
from concourse.bass_utils import run_bass_kernel_spmd  # noqa: E402

B, NS, NQ, DIN, DOUT = 128, 128, 512, 256, 64
NCORES = 8
BL = B // NCORES  # 16 tasks per core
J1, J2 = 15, 17  # Newton iterations for the two 128x128 pivots
F32 = mybir.dt.float32
ALU = mybir.AluOpType
ACTF = mybir.ActivationFunctionType
AXL = mybir.AxisListType
H = 128  # half of DIN; also the partition width


def _newton_inv(nc, cp, sp, pp, Dneg, i128, twoI, onescol, ones1row, one11, iters, tg):
    """Invert SPD D (given as Dneg = -D, [128,128] SBUF) by Newton-Schulz.

    X0 = I/||D||_F, X' = 2X + X @ (-D X). Returns SBUF tile with D^{-1}.
    tg: tag prefix for tile identity across tasks.
    """
    # ||D||_F^2 row partials via (Dneg*1)*Dneg with free-dim accumulate
    frocol = sp.tile([H, 1], F32, name=f"{tg}_frocol", tag=f"{tg}_frocol", bufs=2)
    scr = sp.tile([H, H], F32, name=f"{tg}_froscr", tag=f"{tg}_froscr", bufs=2)
    nc.vector.scalar_tensor_tensor(
        scr[:], Dneg[:], 1.0, Dneg[:], ALU.mult, ALU.mult, accum_out=frocol[:]
    )
    psf = pp.tile([1, 1], F32, name=f"{tg}_psf", tag="ps_tiny", bufs=2)
    nc.tensor.matmul(psf[:], onescol[:], frocol[:], start=True, stop=True)
    fro = sp.tile([1, 1], F32, name=f"{tg}_fro", tag=f"{tg}_fro", bufs=2)
    nc.scalar.activation(fro[:], psf[:], ACTF.Sqrt)  # ||D||_F
    alpha = sp.tile([1, 1], F32, name=f"{tg}_alpha", tag=f"{tg}_alpha", bufs=2)
    nc.vector.reciprocal(alpha[:], fro[:])
    psb = pp.tile([H, 1], F32, name=f"{tg}_psb", tag="ps_tiny", bufs=2)
    nc.tensor.matmul(psb[:], ones1row[:], alpha[:], start=True, stop=True)
    alphav = sp.tile([H, 1], F32, name=f"{tg}_alphav", tag=f"{tg}_alphav", bufs=2)
    nc.scalar.copy(alphav[:], psb[:])

    X = sp.tile([H, H], F32, name=f"{tg}_X0", tag=f"{tg}_X", bufs=6)
    nc.vector.tensor_scalar_mul(X[:], i128[:], alphav[:])
    for k in range(iters):
        psY = pp.tile([H, H], F32, name=f"{tg}_psY", tag="ps", bufs=6)
        nc.tensor.matmul(psY[:], Dneg[:], X[:], start=True, stop=True)
        Yn = sp.tile([H, H], F32, name=f"{tg}_Yn", tag=f"{tg}_Yn", bufs=4)
        nc.scalar.copy(Yn[:], psY[:])  # Yn = -D X
        psX = pp.tile([H, H], F32, name=f"{tg}_psX", tag="ps", bufs=6)
        nc.tensor.matmul(psX[:], X[:], Yn[:], start=True, stop=True)
        Xn = sp.tile([H, H], F32, name=f"{tg}_Xn", tag=f"{tg}_X", bufs=6)
        # X' = 2X + X^T Yn; the asymmetry seed doubles per iter, so
        # resymmetrize every ~6 iterations (cheap, keeps e ~ eps)
        nc.vector.scalar_tensor_tensor(
            Xn[:], X[:], 2.0, psX[:], ALU.mult, ALU.add
        )
        X = Xn
        if k in (6, 12):
            psT = pp.tile([H, H], F32, name=f"{tg}_psT", tag="ps", bufs=6)
            nc.tensor.transpose(psT[:], X[:], i128[:])
            Xs = sp.tile([H, H], F32, name=f"{tg}_Xs", tag=f"{tg}_X", bufs=6)
            nc.vector.tensor_add(Xs[:], X[:], psT[:])
            Xs2 = sp.tile([H, H], F32, name=f"{tg}_Xs2", tag=f"{tg}_X", bufs=6)
            nc.vector.tensor_scalar_mul(Xs2[:], Xs[:], 0.5)
            X = Xs2
    return X


def _build():
    nc = bacc.Bacc("TRN2", target_bir_lowering=False, debug=False)

    phiS_d = nc.dram_tensor("phis", [BL, NS, DIN], F32, kind="ExternalInput")
    yS_d = nc.dram_tensor("ys", [BL, NS, DOUT], F32, kind="ExternalInput")
    phiQ_d = nc.dram_tensor("phiq", [BL, NQ, DIN], F32, kind="ExternalInput")
    yQ_d = nc.dram_tensor("yq", [BL, NQ, DOUT], F32, kind="ExternalInput")
    Ap_d = nc.dram_tensor("aprior", [DIN, DIN], F32, kind="ExternalInput")
    mp_d = nc.dram_tensor("mprior", [DIN, DOUT], F32, kind="ExternalInput")
    eps_d = nc.dram_tensor("sigeps", [1, 1], F32, kind="ExternalInput")
    i128_d = nc.dram_tensor("ceye", [H, H], F32, kind="ExternalInput")
    mask_d = nc.dram_tensor("cmask", [H, DOUT * DOUT], F32, kind="ExternalInput")

    mu_d = nc.dram_tensor("mu_out", [BL, NQ, DOUT], F32, kind="ExternalOutput")
    sig_d = nc.dram_tensor("sig_out", [BL, NQ, DOUT * DOUT], F32, kind="ExternalOutput")
    misc_d = nc.dram_tensor("misc_out", [1, 2], F32, kind="ExternalOutput")

    with tile.TileContext(nc) as tc:
        with (
            tc.tile_pool(name="cp", bufs=1) as cp,
            tc.tile_pool(name="sp", bufs=2) as sp,
            tc.tile_pool(name="pp", bufs=6, space="PSUM") as pp,
        ):
            # ---- constants ----
            i128 = cp.tile([H, H], F32)
            nc.sync.dma_start(i128[:], i128_d[:])
            mask = cp.tile([H, DOUT * DOUT], F32)
            nc.sync.dma_start(mask[:], mask_d[:])
            twoI = cp.tile([H, H], F32)
            nc.vector.tensor_scalar_mul(twoI[:], i128[:], 2.0)
            onescol = cp.tile([H, 1], F32)
            nc.vector.memset(onescol[:], 1.0)
            ones1row = cp.tile([1, H], F32)
            nc.vector.memset(ones1row[:], 1.0)
            one11 = cp.tile([1, 1], F32)
            nc.vector.memset(one11[:], 1.0)
            epssb = cp.tile([1, 1], F32)
            nc.sync.dma_start(epssb[:], eps_d[:])
            psb = pp.tile([H, 1], F32, tag="ps_tiny", bufs=2)
            nc.tensor.matmul(psb[:], ones1row[:], epssb[:], start=True, stop=True)
            epsvec = cp.tile([H, 1], F32)
            nc.scalar.copy(epsvec[:], psb[:])

            # ---- PE warm-up: ~4us of dense bf16 matmuls flips HAM to 8/8 ----
            wb = cp.tile([H, NQ], mybir.dt.bfloat16)
            nc.vector.memset(wb[:], 0.25)
            for wi in range(20):
                psW = pp.tile([H, NQ], F32, name=f"psW{wi}", tag="ps", bufs=6)
                nc.tensor.matmul(psW[:], wb[:, 0:H], wb[:], start=True, stop=True)

            # ---- prior precision P = A A^T and mpn = P m_prior ----
            # load A row-halves, transpose quadrants into AT tiles [j-half, i(256)]
            Ar = []
            for ih in range(2):
                t = cp.tile([H, DIN], F32, name=f"Ar{ih}")
                nc.sync.dma_start(t[:], Ap_d[ih * H : (ih + 1) * H, :])
                Ar.append(t)
            AT = []
            for jh in range(2):
                t = cp.tile([H, DIN], F32, name=f"AT{jh}")
                for ih in range(2):
                    pst = pp.tile([H, H], F32, name=f"psAT{jh}{ih}", tag="ps", bufs=6)
                    nc.tensor.transpose(
                        pst[:], Ar[ih][:, jh * H : (jh + 1) * H], i128[:]
                    )
                    nc.scalar.copy(t[:, ih * H : (ih + 1) * H], pst[:])
                AT.append(t)
            Pt = []
            for ih in range(2):
                psP = pp.tile([H, DIN], F32, name=f"psP{ih}", tag="ps", bufs=6)
                for jh in range(2):
                    nc.tensor.matmul(
                        psP[:],
                        AT[jh][:, ih * H : (ih + 1) * H],
                        AT[jh][:],
                        start=(jh == 0),
                        stop=(jh == 1),
                    )
                t = cp.tile([H, DIN], F32, name=f"Pt{ih}")
                nc.scalar.copy(t[:], psP[:])
                Pt.append(t)
            mpt = []
            for jh in range(2):
                t = cp.tile([H, DOUT], F32, name=f"mpt{jh}")
                nc.sync.dma_start(t[:], mp_d[jh * H : (jh + 1) * H, :])
                mpt.append(t)
            mpn = []
            for ih in range(2):
                psm = pp.tile([H, DOUT], F32, name=f"psmpn{ih}", tag="ps", bufs=6)
                for jh in range(2):
                    nc.tensor.matmul(
                        psm[:],
                        Pt[jh][:, ih * H : (ih + 1) * H],
                        mpt[jh][:],
                        start=(jh == 0),
                        stop=(jh == 1),
                    )
                t = cp.tile([H, DOUT], F32, name=f"mpn{ih}")
                nc.scalar.copy(t[:], psm[:])
                mpn.append(t)

            # batched per-core stats tiles [128, 64]: col = t*4 + qchunk
            spread_all = cp.tile([H, 64], F32)
            spread_eps_all = cp.tile([H, 64], F32)
            resid2_all = cp.tile([H, 64], F32)

            # ---- per-task pipeline ----
            for t in range(BL):
                # Stage A: Gram + rhs
                phiS_t = sp.tile([NS, DIN], F32, tag="phis", bufs=3)
                nc.sync.dma_start(phiS_t[:], phiS_d[t])
                yS_t = sp.tile([NS, DOUT], F32, tag="ysup", bufs=3)
                nc.sync.dma_start(yS_t[:], yS_d[t])

                G = []
                Aneg = sp.tile([H, H], F32, tag="aneg", bufs=3)
                for ih in range(2):
                    psG = pp.tile([H, DIN], F32, name=f"psG{ih}", tag="ps", bufs=6)
                    nc.tensor.matmul(
                        psG[:],
                        phiS_t[:, ih * H : (ih + 1) * H],
                        phiS_t[:],
                        start=True,
                        stop=True,
                    )
                    g = sp.tile([H, DIN], F32, name=f"G{ih}", tag=f"g{ih}", bufs=3)
                    nc.vector.tensor_add(g[:], psG[:], Pt[ih][:])
                    if ih == 0:
                        # Aneg = -(psG + P)[:, :128]
                        nc.vector.scalar_tensor_tensor(
                            Aneg[:],
                            psG[:, 0:H],
                            -1.0,
                            Pt[0][:, 0:H],
                            ALU.mult,
                            ALU.subtract,
                        )
                    G.append(g)
                V = []
                for ih in range(2):
                    psR = pp.tile([H, DOUT], F32, name=f"psR{ih}", tag="ps", bufs=6)
                    nc.tensor.matmul(
                        psR[:],
                        phiS_t[:, ih * H : (ih + 1) * H],
                        yS_t[:],
                        start=True,
                        stop=True,
                    )
                    v = sp.tile([H, DOUT], F32, name=f"V{ih}", tag=f"v{ih}", bufs=3)
                    nc.vector.tensor_add(v[:], psR[:], mpn[ih][:])
                    V.append(v)

                # Stage B: invert A, build U/UT/Schur, invert Schur
                Ainv = _newton_inv(
                    nc, cp, sp, pp, Aneg, i128, twoI, onescol, ones1row, one11, J1, "na"
                )
                Bv = G[0][:, H:DIN]  # B block view [128, 128]
                psU = pp.tile([H, H], F32, tag="ps", bufs=6)
                nc.tensor.matmul(psU[:], Ainv[:], Bv, start=True, stop=True)
                U = sp.tile([H, H], F32, tag="u", bufs=3)
                nc.scalar.copy(U[:], psU[:])
                psUT = pp.tile([H, H], F32, tag="ps", bufs=6)
                nc.tensor.matmul(psUT[:], Bv, Ainv[:], start=True, stop=True)
                UT = sp.tile([H, H], F32, tag="ut", bufs=3)
                nc.scalar.copy(UT[:], psUT[:])
                psSc = pp.tile([H, H], F32, tag="ps", bufs=6)
                nc.tensor.matmul(psSc[:], Bv, U[:], start=True, stop=True)
                Scneg0 = sp.tile([H, H], F32, tag="scneg0", bufs=3)
                # Scneg = B^T U - C = -(C - B^T U), then symmetrized (the
                # Newton recurrence needs D^T = D bitwise)
                nc.vector.tensor_sub(Scneg0[:], psSc[:], G[1][:, H:DIN])
                psScT = pp.tile([H, H], F32, tag="ps", bufs=6)
                nc.tensor.transpose(psScT[:], Scneg0[:], i128[:])
                Sctmp = sp.tile([H, H], F32, tag="sctmp", bufs=3)
                nc.vector.tensor_add(Sctmp[:], Scneg0[:], psScT[:])
                Scneg = sp.tile([H, H], F32, tag="scneg", bufs=3)
                nc.vector.tensor_scalar_mul(Scneg[:], Sctmp[:], 0.5)
                Scinv = _newton_inv(
                    nc, cp, sp, pp, Scneg, i128, twoI, onescol, ones1row, one11, J2, "ns"
                )

                # Stage C: m = S rhs with one refinement pass, factored applies
                def apply_s(V1ap, V2ap, nm):
                    psT2 = pp.tile([H, DOUT], F32, name=f"psT2{nm}", tag="ps", bufs=6)
                    nc.tensor.matmul(psT2[:], U[:], V1ap, start=True, stop=True)
                    T2 = sp.tile([H, DOUT], F32, name=f"T2{nm}", tag=f"t2{nm}", bufs=2)
                    nc.vector.tensor_sub(T2[:], V2ap, psT2[:])
                    psZ1 = pp.tile([H, DOUT], F32, name=f"psZ1{nm}", tag="ps", bufs=6)
                    nc.tensor.matmul(psZ1[:], Ainv[:], V1ap, start=True, stop=True)
                    Z1 = sp.tile([H, DOUT], F32, name=f"Z1{nm}", tag=f"z1{nm}", bufs=2)
                    nc.scalar.copy(Z1[:], psZ1[:])
                    psZ2 = pp.tile([H, DOUT], F32, name=f"psZ2{nm}", tag="ps", bufs=6)
                    nc.tensor.matmul(psZ2[:], Scinv[:], T2[:], start=True, stop=True)
                    Z2 = sp.tile([H, DOUT], F32, name=f"Z2{nm}", tag=f"z2{nm}", bufs=2)
                    nc.scalar.copy(Z2[:], psZ2[:])
                    psR1 = pp.tile([H, DOUT], F32, name=f"psR1{nm}", tag="ps", bufs=6)
                    nc.tensor.matmul(psR1[:], UT[:], Z2[:], start=True, stop=True)
                    M1 = sp.tile([H, DOUT], F32, name=f"M1{nm}", tag=f"m1{nm}", bufs=2)
                    nc.vector.tensor_sub(M1[:], Z1[:], psR1[:])
                    return M1, Z2

                m1, m2 = apply_s(V[0][:], V[1][:], "a")
                mm = [m1, m2]
                R = []
                for ih in range(2):
                    psGm = pp.tile([H, DOUT], F32, name=f"psGm{ih}", tag="ps", bufs=6)
                    for jh in range(2):
                        nc.tensor.matmul(
                            psGm[:],
                            G[jh][:, ih * H : (ih + 1) * H],
                            mm[jh][:],
                            start=(jh == 0),
                            stop=(jh == 1),
                        )
                    r = sp.tile([H, DOUT], F32, name=f"R{ih}", tag=f"r{ih}", bufs=2)
                    nc.vector.tensor_sub(r[:], V[ih][:], psGm[:])
                    R.append(r)
                d1, d2 = apply_s(R[0][:], R[1][:], "b")
                mf1 = sp.tile([H, DOUT], F32, tag="mf1", bufs=2)
                nc.vector.tensor_add(mf1[:], m1[:], d1[:])
                mf2 = sp.tile([H, DOUT], F32, tag="mf2", bufs=2)
                nc.vector.tensor_add(mf2[:], m2[:], d2[:])
                mf = [mf1, mf2]

                # Stage D: phiQ transpose, mu/resid, spread
                natQ = sp.tile([H, 4 * DIN], F32, tag="natq", bufs=2)
                yQn = sp.tile([H, 4 * DOUT], F32, tag="yqn", bufs=2)
                for c in range(4):
                    nc.sync.dma_start(
                        natQ[:, c * DIN : (c + 1) * DIN],
                        phiQ_d[t, c * H : (c + 1) * H, :],
                    )
                    nc.sync.dma_start(
                        yQn[:, c * DOUT : (c + 1) * DOUT],
                        yQ_d[t, c * H : (c + 1) * H, :],
                    )
                F1 = sp.tile([H, NQ], F32, tag="f1", bufs=3)
                F2 = sp.tile([H, NQ], F32, tag="f2", bufs=3)
                for c in range(4):
                    for ih, Ft in ((0, F1), (1, F2)):
                        pst = pp.tile(
                            [H, H], F32, name=f"psFt{c}{ih}", tag="ps", bufs=6
                        )
                        nc.tensor.transpose(
                            pst[:],
                            natQ[:, c * DIN + ih * H : c * DIN + (ih + 1) * H],
                            i128[:],
                        )
                        nc.scalar.copy(Ft[:, c * H : (c + 1) * H], pst[:])

                mu_st = sp.tile([H, 4 * DOUT], F32, tag="must", bufs=2)
                sqscr = sp.tile([H, DOUT], F32, tag="sqscr", bufs=2)
                for c in range(4):
                    psMu = pp.tile([H, DOUT], F32, name=f"psMu{c}", tag="ps", bufs=6)
                    for ih in range(2):
                        nc.tensor.matmul(
                            psMu[:],
                            (F1 if ih == 0 else F2)[:, c * H : (c + 1) * H],
                            mf[ih][:],
                            start=(ih == 0),
                            stop=(ih == 1),
                        )
                    nc.scalar.copy(mu_st[:, c * DOUT : (c + 1) * DOUT], psMu[:])
                    resid = sp.tile([H, DOUT], F32, name=f"resid{c}", tag="resid", bufs=2)
                    nc.vector.tensor_sub(
                        resid[:], yQn[:, c * DOUT : (c + 1) * DOUT], psMu[:]
                    )
                    col = t * 4 + c
                    nc.scalar.activation(
                        sqscr[:],
                        resid[:],
                        ACTF.Square,
                        accum_out=resid2_all[:, col : col + 1],
                    )
                for c in range(4):
                    nc.sync.dma_start(
                        mu_d[t, c * H : (c + 1) * H, :],
                        mu_st[:, c * DOUT : (c + 1) * DOUT],
                    )

                # spread: z-form  s = (Ainv F1) . F1 + (Scinv H) . H,  H = F2 - U^T F1
                psH = pp.tile([H, NQ], F32, tag="ps", bufs=6)
                nc.tensor.matmul(psH[:], U[:], F1[:], start=True, stop=True)
                Ht = sp.tile([H, NQ], F32, tag="ht", bufs=2)
                nc.vector.tensor_sub(Ht[:], F2[:], psH[:])
                psY1 = pp.tile([H, NQ], F32, tag="ps", bufs=6)
                nc.tensor.matmul(psY1[:], Ainv[:], F1[:], start=True, stop=True)
                s1 = sp.tile([H, NQ], F32, tag="s1", bufs=2)
                nc.vector.tensor_mul(s1[:], psY1[:], F1[:])
                psY2 = pp.tile([H, NQ], F32, tag="ps", bufs=6)
                nc.tensor.matmul(psY2[:], Scinv[:], Ht[:], start=True, stop=True)
                s2 = sp.tile([H, NQ], F32, tag="s2", bufs=2)
                nc.vector.tensor_mul(s2[:], psY2[:], Ht[:])
                psSp = pp.tile([1, NQ], F32, tag="ps", bufs=6)
                nc.tensor.matmul(psSp[:], onescol[:], s1[:], start=True, stop=False)
                nc.tensor.matmul(psSp[:], onescol[:], s2[:], start=False, stop=True)
                sprow = sp.tile([1, NQ], F32, tag="sprow", bufs=2)
                nc.scalar.copy(sprow[:], psSp[:])
                for c in range(4):
                    col = t * 4 + c
                    psq = pp.tile([H, 1], F32, name=f"psq{c}", tag="ps_tiny", bufs=2)
                    nc.tensor.matmul(
                        psq[:],
                        sprow[:, c * H : (c + 1) * H],
                        one11[:],
                        start=True,
                        stop=True,
                    )
                    # spread = qSq + 1
                    nc.scalar.activation(
                        spread_all[:, col : col + 1], psq[:], ACTF.Copy, bias=1.0
                    )
                    nc.vector.tensor_scalar_mul(
                        spread_eps_all[:, col : col + 1],
                        spread_all[:, col : col + 1],
                        epsvec[:],
                    )
                    # sig block: [128 q, 64*64] = mask * spread_eps
                    sig_t = sp.tile([H, DOUT * DOUT], F32, name=f"sig{c}", tag="sig", bufs=3)
                    nc.vector.tensor_scalar_mul(
                        sig_t[:], mask[:], spread_eps_all[:, col : col + 1]
                    )
                    nc.sync.dma_start(sig_d[t, c * H : (c + 1) * H, :], sig_t[:])

            # ---- Stage E: nll partials ----
            logt = cp.tile([H, 64], F32)
            nc.scalar.activation(logt[:], spread_all[:], ACTF.Ln)
            recip = cp.tile([H, 64], F32)
            nc.vector.reciprocal(recip[:], spread_eps_all[:])
            quad = cp.tile([H, 64], F32)
            nc.vector.tensor_mul(quad[:], resid2_all[:], recip[:])
            lq = cp.tile([H, 2], F32)
            nc.vector.tensor_reduce(lq[:, 0:1], logt[:], AXL.X, ALU.add)
            nc.vector.tensor_reduce(lq[:, 1:2], quad[:], AXL.X, ALU.add)
            psF = pp.tile([1, 2], F32, tag="ps_tiny", bufs=2)
            nc.tensor.matmul(psF[:], onescol[:], lq[:], start=True, stop=True)
            misc = cp.tile([1, 2], F32)
            nc.scalar.copy(misc[:], psF[:])
            nc.sync.dma_start(misc_d[:], misc[:])

    nc.compile()
    return nc


_NC_CACHE = []


def _get_nc():
    if not _NC_CACHE:
        _NC_CACHE.append(_build())
    return _NC_CACHE[0]


def kernel(phi_support, y_support, phi_query, y_query, m_prior, S_inv_prior_asym, sig_eps):
    f = np.float32
    phi_support = np.ascontiguousarray(phi_support, dtype=f)
    y_support = np.ascontiguousarray(y_support, dtype=f)
    phi_query = np.ascontiguousarray(phi_query, dtype=f)
    y_query = np.ascontiguousarray(y_query, dtype=f)
    m_prior = np.ascontiguousarray(m_prior, dtype=f)
    A = np.ascontiguousarray(S_inv_prior_asym, dtype=f)
    eps = np.asarray(sig_eps, dtype=f).reshape(1, 1)

    eye = np.eye(H, dtype=f)
    mask = np.zeros((H, DOUT * DOUT), dtype=f)
    mask[:, :: DOUT + 1] = 1.0  # flattened 64x64 identity pattern per partition

    in_maps = []
    for c in range(NCORES):
        s = slice(c * BL, (c + 1) * BL)
        in_maps.append(
            {
                "phis": phi_support[s],
                "ys": y_support[s],
                "phiq": phi_query[s],
                "yq": y_query[s],
                "aprior": A,
                "mprior": m_prior,
                "sigeps": eps,
                "ceye": eye,
                "cmask": mask,
            }
        )

    nc = _get_nc()
    res = run_bass_kernel_spmd(nc, in_maps, core_ids=list(range(NCORES)), trace=True)
    if res.exec_time_ns is not None:
        print(f"HW exec time: {res.exec_time_ns} ns")
        kernel.last_exec_time_ns = res.exec_time_ns

    mu = np.empty((B, NQ, DOUT), dtype=f)
    sig = np.empty((B, NQ, DOUT, DOUT), dtype=f)
    s_log = 0.0
    s_quad = 0.0
    for c in range(NCORES):
        out = res.results[c]
        mu[c * BL : (c + 1) * BL] = out["mu_out"]
        sig[c * BL : (c + 1) * BL] = out["sig_out"].reshape(BL, NQ, DOUT, DOUT)
        s_log += float(out["misc_out"][0, 0])
        s_quad += float(out["misc_out"][0, 1])
    BQ = B * NQ
    nll = np.float32(DOUT * (s_log / BQ + np.log(float(eps[0, 0]))) + s_quad / BQ)
    return mu, sig, nll
